# revision 28
# baseline (speedup 1.0000x reference)
"""Trainium2 Bass kernel for the ADMM total-variation solver (nn_ADMM).

Math: x <- B^-1(bA + v) iterated 50x, B = AtA + g*DtD + a*I.  AtA is
rank-9 and C := g*DtD + a*I is circulant, so by Woodbury
    B^-1 = C^-1 - W2 U^T,   U = C^-1 A^T,  W2 = U S^-1,  S = I9 + A U.
C^-1 is applied as a banded (radius-32) circular convolution G; the
rank-9 correction uses q = U^T v.  All 8 cores run the same program
(SPMD, no collectives); core 0's output is returned.

Perf design (vs the fp32 baseline):
- every matmul is bf16 (4x PE throughput).  A1/B/I/ones have exact bf16
  entries; G is split hi+lo bf16 (two matmuls ~= fp16 operator
  precision), which kills the systematic operator-rounding error that a
  plain bf16 G would accumulate over 50 non-contracting iterations.
- At = A1 x + B x + (E - U) accumulates entirely in one PSUM bank on PE
  (an I-matmul folds the state term), so the soft-threshold reads PSUM
  directly and the En/At DVE adds leave the critical path.
- no halo columns: each banded operator's cross-column corner is applied
  as two column-shifted matmuls on the payload tile itself (out cols
  1:32 from src cols 0:31 plus the single wrap column), so nothing waits
  on halo maintenance copies.
- the tau-side state algebra is restructured so only two fused DVE ops
  (Dt = 2*Cx + tTW, P2n = -bankB - (Cx + S2)) sit before v; the state
  recurrences (tTW' = Cx + tTW - Wn, tAB' = En - Un, S2' = tAB' + tTW')
  run on GpSimd during the rank-9 tail of the same iteration.
- rank-9: Z1 = U2 (.) v as one bf16 multiply; the c-grouped sums go
  through Pool avg (window scales baked into W2 host-side, bf16 output,
  no fp32-only restriction -> no cast); a ones-matmul does the partition
  reduce AND the broadcast of q; c0 = B^-1 bA rides along as a 10th
  rank column with q10 = -1.
- x and v tiles are bf16; small bf16 filler matmuls keep the PE
  pipeline warm across its idle windows.

Vector layout: [128, 32] tiles, flat index i = k + 128*c at tile col c.
"""

import numpy as np

N = 4096
P = 128          # partitions
CCOL = 32        # payload columns; i = k + 128*c
RB = 32          # band radius of G
R9 = 9           # Woodbury rank
R10 = 10         # rank columns incl. the c0 slot
GAMMA = 10.0
ALPHA = 5.0
LAM = 1e-4
NIT = 50
NCORES = 8
USE_POOL = False  # Pool-avg grouped sums (False: tensor_reduce + cast)

# f32-column offsets inside the constant blob [128, BLOB_COLS].
# bf16 payloads are packed two-per-f32-column and bitcast on device.
_cur = 0
def _alloc(w):
    global _cur
    off = _cur
    _cur += w
    return off

OFF_A1M   = _alloc(64)    # gamma*(S+ - I) main, bf16 [128,128]
OFF_BM    = _alloc(64)    # gamma*(S- - I) main
OFF_ABM   = _alloc(64)    # gamma*(S+ + S- - 2I) main (A1_M + B_M)
OFF_IBF   = _alloc(64)    # identity (tAB fold into the At bank)
OFF_A1C   = _alloc(64)    # A1 corner (single element, padded)
OFF_BC    = _alloc(64)    # B corner
OFF_GHIM  = _alloc(64)    # C^-1 banded main, hi half
OFF_GLOM  = _alloc(64)    # lo half
OFF_GBLH  = _alloc(64)    # C^-1 left corner hi
OFF_GBLL  = _alloc(64)
OFF_GBHH  = _alloc(64)    # C^-1 right corner hi
OFF_GBHL  = _alloc(64)
OFF_ONES  = _alloc(64)    # all-ones bf16 (partition reduce + broadcast)
OFF_U2    = _alloc(144)   # U m-major: [k, m*32+c], bf16 [128, 288]
OFF_W2    = _alloc(160)   # [W2 | c0] c-major: [k, c*10+m], bf16
OFF_X0    = _alloc(16)    # x0 tile, bf16 [128, 32]
BLOB_COLS = _cur


def _bf16(x):
    x32 = np.asarray(x, np.float32)
    u = x32.view(np.uint32)
    r = ((u >> 16) + ((u >> 15) & 1)).astype(np.uint32) << 16
    return r.view(np.float32)


def _pack_bf16(arr):
    """[128, W] float (W even) -> [128, W//2] f32 with packed bf16 pairs."""
    a = _bf16(arr).view(np.uint32) >> 16
    lo, hi = a[:, 0::2], a[:, 1::2]
    return (lo | (hi << 16)).view(np.float32)


def _banded(h):
    """main/BL/BH lhsT pieces for kernel h (dict d -> coef), [128,128] each.
    lhsT[k, m]: contraction index k = input row, m = output row.
    main: within-column (shift d = k - m);
    BL: rhs = col c-1 view (shift d = k - 128 - m);
    BH: rhs = col c+1 view (shift d = k + 128 - m)."""
    B0 = np.zeros((P, P)); BL = np.zeros((P, P)); BH = np.zeros((P, P))
    for k in range(P):
        for m in range(P):
            if (k - m) in h:
                B0[k, m] = h[k - m]
            if (k - P - m) in h:
                BL[k, m] = h[k - P - m]
            if (k + P - m) in h:
                BH[k, m] = h[k + P - m]
    return B0, BL, BH


def _tile(vec):
    """[4096] -> [128, 32], i = k + 128*c."""
    return np.ascontiguousarray(np.asarray(vec).reshape(CCOL, P).T)


def host_constants(target, A, x0):
    """All f64 precompute; returns the [128, BLOB_COLS] f32 device blob."""
    A64 = np.asarray(A, np.float64)
    w = ALPHA + 2 * GAMMA * (1 - np.cos(2 * np.pi * np.arange(N // 2 + 1) / N))

    def C_inv(z):
        return np.fft.irfft(np.fft.rfft(z, axis=-1) / w, n=N, axis=-1)

    U = C_inv(A64).T                              # [N, 9]
    S = np.eye(R9) + A64 @ U
    W2 = U @ np.linalg.inv(S)                     # [N, 9]
    g = np.fft.irfft(1.0 / w, n=N)                # kernel of C^-1
    b = A64 @ np.asarray(target, np.float64)
    bA = b @ A64
    c0 = C_inv(bA) - W2 @ (U.T @ bA)              # B^-1 @ bA

    blob = np.zeros((P, BLOB_COLS), np.float32)

    def putb(off, arr):
        p = _pack_bf16(arr)
        blob[:p.shape[0], off:off + p.shape[1]] = p

    mA1 = _banded({-1: GAMMA, 0: -GAMMA})         # gamma*(S+ - I)
    mB = _banded({1: GAMMA, 0: -GAMMA})           # gamma*(S- - I)
    mG = _banded({d: g[d % N] for d in range(-RB, RB + 1)})
    Ghi = [_bf16(m).astype(np.float64) for m in mG]
    Glo = [m - h for m, h in zip(mG, Ghi)]

    putb(OFF_A1M, mA1[0]); putb(OFF_A1C, mA1[1])
    putb(OFF_BM, mB[0]); putb(OFF_BC, mB[2])
    putb(OFF_ABM, mA1[0] + mB[0])
    putb(OFF_GHIM, Ghi[0]); putb(OFF_GLOM, Glo[0])
    putb(OFF_GBLH, Ghi[1]); putb(OFF_GBLL, Glo[1])
    putb(OFF_GBHH, Ghi[2]); putb(OFF_GBHL, Glo[2])
    putb(OFF_IBF, np.eye(P))
    putb(OFF_ONES, np.ones((P, P)))

    # U2[k, m*32+c] = U[k + 128c, m] (m-major)
    putb(OFF_U2, U.reshape(CCOL, P, R9).transpose(1, 2, 0).reshape(P, R9 * CCOL))
    # W2e[k, c*10+m] = W2[k + 128c, m] for m<9;  c0[k + 128c] at m=9.
    # Pool avg divides the c-sum by 32 and the m-sum by 10: bake the
    # compensation into the constants (q10 = -1 skips the first pool).
    if USE_POOL:
        W2e = np.concatenate([W2 * (CCOL * R10), c0[:, None] * R10], axis=1)
    else:
        W2e = np.concatenate([W2, c0[:, None]], axis=1)
    putb(OFF_W2, W2e.reshape(CCOL, P, R10).transpose(1, 0, 2).reshape(P, CCOL * R10))

    putb(OFF_X0, _tile(np.asarray(x0, np.float64)))
    return np.ascontiguousarray(blob)


def build_nc():
    """Build and compile the Bacc graph (one core's program)."""
    from concourse import bacc, mybir, tile

    f32 = mybir.dt.float32
    bf16 = mybir.dt.bfloat16
    Relu = mybir.ActivationFunctionType.Relu
    Copy = mybir.ActivationFunctionType.Copy
    Alu = mybir.AluOpType
    Avg = mybir.PoolFunctionType.avg
    nc = bacc.Bacc(target_bir_lowering=False)

    blob_ext = nc.declare_dram_parameter("blob", [P, BLOB_COLS], f32, isOutput=False)
    out_ext = nc.declare_dram_parameter("out", [P, CCOL], f32, isOutput=True)

    with tile.TileContext(nc) as tc:
        with (
            tc.tile_pool(name="const", bufs=1) as cpool,
            tc.tile_pool(name="work", bufs=3) as wpool,
            tc.tile_pool(name="psum", bufs=1, space="PSUM") as ppool,
        ):
            cb = cpool.tile([P, BLOB_COLS], f32, tag="blob")
            nc.sync.dma_start(cb[:, :], blob_ext[:, :])

            def csb(off, wcols):
                """bf16 view of wcols f32 columns -> [128, 2*wcols] bf16"""
                return cb[:, off:off + wcols].bitcast(bf16)

            A1_M, A1_C = csb(OFF_A1M, 64), csb(OFF_A1C, 64)
            B_M, B_C = csb(OFF_BM, 64), csb(OFF_BC, 64)
            AB_M = csb(OFF_ABM, 64)
            I_bf = csb(OFF_IBF, 64)
            GHI_M, GLO_M = csb(OFF_GHIM, 64), csb(OFF_GLOM, 64)
            GBL_H, GBL_L = csb(OFF_GBLH, 64), csb(OFF_GBLL, 64)
            GBH_H, GBH_L = csb(OFF_GBHH, 64), csb(OFF_GBHL, 64)
            ones_bf = csb(OFF_ONES, 64)
            U2 = csb(OFF_U2, 144)                  # [128, 288] bf16
            U2_3d = U2.rearrange("k (m c) -> k m c", c=CCOL)
            W2e = csb(OFF_W2, 160)                 # [128, 320] bf16
            W2_3d = W2e.rearrange("k (c m) -> k c m", m=R10)

            def pool_avg(out_ap, in3d):
                """Grouped average via InstPool.  The input AP is lowered
                un-optimized with exactly 5 dims so the window (innermost
                dim) survives to codegen."""
                ap5 = in3d.unsqueeze(1).unsqueeze(1)
                return nc.vector.add_instruction(mybir.InstPool(
                    name=f"I-{nc.vector.bass.next_id()}",
                    func=Avg,
                    ins=[nc.vector.lower_ap(ap5, opt=False)],
                    outs=[nc.vector.lower_ap(out_ap)],
                ))

            def corner(bank, lhsT, src, shift, stop=False):
                """Cross-column corner of a banded circulant: out col c
                reads src col c+shift (mod 32), as two payload matmuls."""
                if shift == -1:
                    nc.tensor.matmul(bank[:, 1:CCOL], lhsT, src[:, 0:CCOL - 1],
                                     start=False, stop=False,
                                     skip_group_check=True)
                    nc.tensor.matmul(bank[:, 0:1], lhsT, src[:, CCOL - 1:CCOL],
                                     start=False, stop=stop,
                                     skip_group_check=True)
                else:
                    nc.tensor.matmul(bank[:, 0:CCOL - 1], lhsT, src[:, 1:CCOL],
                                     start=False, stop=False,
                                     skip_group_check=True)
                    nc.tensor.matmul(bank[:, CCOL - 1:CCOL], lhsT, src[:, 0:1],
                                     start=False, stop=stop,
                                     skip_group_check=True)

            # q partials [128, 10]: cols 0:9 rewritten each iteration,
            # col 9 = -1/128 so the ones-matmul yields q10 = -1 (c0 slot).
            Z1r = cpool.tile([P, 16], bf16, tag="Z1r")
            nc.vector.memset(Z1r[:, :], 0.0)
            nc.vector.memset(Z1r[:, R9:R9 + 1], -1.0 / P)

            Xh = csb(OFF_X0, 16)                   # [128, 32] bf16 state
            tABf = tABb = tTW = S2 = None

            for j in range(1, NIT + 1):
                first = (j == 1)
                last = (j == NIT)
                xm = Xh[:, 0:CCOL]

                # --- PE: At bank (A1 x + B x + tAB) and B bank ---
                bankAt = ppool.tile([P, CCOL], f32, tag="At")
                bankB = ppool.tile([P, CCOL], f32, tag="B")
                scr = ppool.tile([P, 64], f32, tag="scr")
                if first:
                    nc.tensor.matmul(bankAt[:, :], A1_M, xm, start=True, stop=False)
                    corner(bankAt, A1_C, xm, -1, stop=True)
                else:
                    nc.tensor.matmul(bankAt[:, :], AB_M, xm, start=True, stop=False)
                    corner(bankAt, A1_C, xm, -1)
                    corner(bankAt, B_C, xm, +1)
                    nc.tensor.matmul(bankAt[:, :], I_bf, tABb[:, :], start=False, stop=True)
                nc.tensor.matmul(bankB[:, :], B_M, xm, start=True, stop=False)
                corner(bankB, B_C, xm, +1, stop=True)
                # keep the PE pipeline warm while DVE runs the soft chain
                nc.tensor.matmul(scr[:, :], ones_bf, ones_bf[:, 0:64],
                                 start=True, stop=True)
                nc.tensor.matmul(scr[:, :], ones_bf, ones_bf[:, 0:64],
                                 start=True, stop=True)

                # --- Scalar: Cx = alpha*x ---
                if first:
                    Wn = wpool.tile([P, CCOL], f32, tag="Wn")
                    nc.scalar.activation(Wn[:, :], xm, Relu, bias=0.0, scale=ALPHA)
                else:
                    Cx = wpool.tile([P, CCOL], f32, tag="Cx")
                    nc.scalar.activation(Cx[:, :], xm, Copy, bias=0.0, scale=ALPHA)
                    # GpSimd: u3 = Cx + S2 feeds P2n
                    u3 = wpool.tile([P, CCOL], f32, tag="u3")
                    nc.gpsimd.tensor_add(u3[:, :], Cx[:, :], S2[:, :])

                # --- DVE: Dt, soft-threshold, v ---
                r1 = wpool.tile([P, CCOL], f32, tag="r1")
                Un = wpool.tile([P, CCOL], f32, tag="Un")
                vh = wpool.tile([P, CCOL], bf16, tag="vh")
                vm = vh[:, 0:CCOL]
                if not first:
                    Dt = wpool.tile([P, CCOL], f32, tag="Dt")
                    # Dt = 2*Cx + tTW  (= alpha*x + tau)
                    nc.vector.scalar_tensor_tensor(Dt[:, :], Cx[:, :], 2.0,
                                                   tTW[:, :], Alu.mult, Alu.add)
                nc.vector.tensor_scalar(r1[:, :], bankAt[:, :], -LAM, LAM,
                                        Alu.max, Alu.min)
                nc.vector.tensor_sub(Un[:, :], bankAt[:, :], r1[:, :])
                if first:
                    nc.vector.tensor_add(vm, Un[:, :], Wn[:, :])
                else:
                    P1 = wpool.tile([P, CCOL], f32, tag="P1")
                    P2n = wpool.tile([P, CCOL], f32, tag="P2n")
                    # P1 = relu(Dt) + Un;  P2n = -bankB - u3 = -(En + Tn)
                    nc.vector.scalar_tensor_tensor(P1[:, :], Dt[:, :], 0.0,
                                                   Un[:, :], Alu.max, Alu.add)
                    nc.vector.scalar_tensor_tensor(P2n[:, :], bankB[:, :], -1.0,
                                                   u3[:, :], Alu.mult,
                                                   Alu.subtract)
                    nc.vector.tensor_add(vm, P1[:, :], P2n[:, :])
                    # Scalar: Wn = relu(Dt) (state only)
                    Wn = wpool.tile([P, CCOL], f32, tag="Wn")
                    nc.scalar.activation(Wn[:, :], Dt[:, :], Relu,
                                         bias=0.0, scale=1.0)

                # --- PE: banded G apply (mains + halo-free corners) ---
                bankE = ppool.tile([P, CCOL], f32, tag="E")
                bankR = ppool.tile([P, 16], f32, tag="R")
                nc.tensor.matmul(bankE[:, :], GHI_M, vm, start=True, stop=False)
                nc.tensor.matmul(bankE[:, :], GLO_M, vm, start=False, stop=False)
                corner(bankE, GBL_H, vm, -1)
                corner(bankE, GBL_L, vm, -1)
                corner(bankE, GBH_H, vm, +1)
                corner(bankE, GBH_L, vm, +1, stop=True)

                # --- DVE: rank-9 q = U^T v ---
                Z1 = wpool.tile([P, R9 * CCOL], bf16, tag="Z1")
                z1_3d = Z1[:, :].rearrange("k (m c) -> k m c", c=CCOL)
                vb9 = vm.unsqueeze(1).broadcast_to([P, R9, CCOL])
                nc.vector.tensor_mul(z1_3d, U2_3d, vb9)
                if USE_POOL:
                    pool_avg(Z1r[:, 0:R9], z1_3d)
                    # PE: partition reduce + broadcast of q (and q10 = -1)
                    nc.tensor.matmul(bankR[:, 0:R10], ones_bf, Z1r[:, 0:R10],
                                     start=True, stop=True)
                else:
                    with nc.allow_low_precision(reason="q is consumed in bf16"):
                        nc.vector.tensor_reduce(Z1r[:, 0:R9], z1_3d,
                                                axis=mybir.AxisListType.X,
                                                op=Alu.add)
                    nc.tensor.matmul(bankR[:, 0:R10], ones_bf, Z1r[:, 0:R10],
                                     start=True, stop=True)
                nc.tensor.matmul(scr[:, :], ones_bf, ones_bf[:, 0:64],
                                 start=True, stop=True)

                # --- DVE: corr = W2e q (incl. -c0) and the x update ---
                qb = wpool.tile([P, 16], bf16, tag="qb")
                nc.vector.tensor_copy(qb[:, 0:R10], bankR[:, 0:R10])
                Z2 = wpool.tile([P, CCOL * R10], bf16, tag="Z2")
                corr = wpool.tile([P, CCOL], bf16, tag="corr")
                z2_3d = Z2[:, :].rearrange("k (c m) -> k c m", m=R10)
                rb = qb[:, 0:R10].unsqueeze(1).broadcast_to([P, CCOL, R10])
                nc.vector.tensor_mul(z2_3d, W2_3d, rb)
                if USE_POOL:
                    pool_avg(corr[:, :], z2_3d)
                else:
                    with nc.allow_low_precision(reason="x state is bf16"):
                        nc.vector.tensor_reduce(corr[:, :], z2_3d,
                                                axis=mybir.AxisListType.X,
                                                op=Alu.add)


                if not last:
                    Xn = wpool.tile([P, CCOL], bf16, tag="Xh")
                    nc.vector.tensor_sub(Xn[:, :], bankE[:, :], corr[:, :])
                    Xh = Xn

                    # next-iteration state combos (GpSimd/Scalar, in the tail)
                    tABf_n = wpool.tile([P, CCOL], f32, tag="tABf")
                    tTW_n = wpool.tile([P, CCOL], f32, tag="tTW")
                    S2_n = wpool.tile([P, CCOL], f32, tag="S2")
                    tABb_n = wpool.tile([P, CCOL], bf16, tag="tABb")
                    if first:
                        nc.gpsimd.tensor_scalar_mul(tABf_n[:, :], Un[:, :], -1.0)
                        nc.gpsimd.tensor_scalar_mul(tTW_n[:, :], Wn[:, :], -1.0)
                    else:
                        # En = bankB + tAB reconstructed off-PSUM:
                        # P2n = -bankB - u3  =>  En = tAB - P2n - u3
                        e1 = wpool.tile([P, CCOL], f32, tag="e1")
                        En = wpool.tile([P, CCOL], f32, tag="En")
                        nc.gpsimd.tensor_sub(e1[:, :], tABf[:, :], P2n[:, :])
                        nc.gpsimd.tensor_sub(En[:, :], e1[:, :], u3[:, :])
                        nc.gpsimd.tensor_sub(tABf_n[:, :], En[:, :], Un[:, :])
                        # tTW' = Tn - Wn = (Cx + tTW) - Wn
                        tmpW = wpool.tile([P, CCOL], f32, tag="tmpW")
                        nc.gpsimd.tensor_sub(tmpW[:, :], Cx[:, :], Wn[:, :])
                        nc.gpsimd.tensor_add(tTW_n[:, :], tmpW[:, :], tTW[:, :])
                    nc.gpsimd.tensor_add(S2_n[:, :], tABf_n[:, :], tTW_n[:, :])
                    nc.scalar.activation(tABb_n[:, :], tABf_n[:, :], Copy,
                                         bias=0.0, scale=1.0)
                    tABf, tTW, S2, tABb = tABf_n, tTW_n, S2_n, tABb_n
                else:
                    Xout = wpool.tile([P, CCOL], f32, tag="Xout")
                    nc.vector.tensor_sub(Xout[:, :], bankE[:, :], corr[:, :])
                    nc.sync.dma_start(out_ext[:, :], Xout[:, :])
                    # touch the filler bank so dead-code elim keeps the
                    # pipeline-warming matmuls
                    scrk = wpool.tile([P, 1], f32, tag="scrk")
                    nc.vector.tensor_copy(scrk[:, :], scr[:, 0:1])
                    nc.sync.dma_start(out_ext[0:1, 0:1], scrk[0:1, 0:1])

    nc.compile()
    return nc


def kernel(**inputs):
    from concourse.bass_utils import run_bass_kernel_spmd

    target = np.asarray(inputs["target"], np.float32)
    A = np.asarray(inputs["A"], np.float32)
    x0 = np.asarray(inputs["x0"], np.float32)

    blob = host_constants(target, A, x0)
    nc = build_nc()
    in_maps = [{"blob": blob} for _ in range(NCORES)]
    res = run_bass_kernel_spmd(nc, in_maps, core_ids=list(range(NCORES)))
    out_tile = np.asarray(res.results[0]["out"], np.float32)
    return np.ascontiguousarray(out_tile.T.reshape(-1))


# revision 30
# speedup vs baseline: 1.0064x; 1.0064x over previous
"""Trainium2 Bass kernel for the ADMM total-variation solver (nn_ADMM).

Math: x <- B^-1(bA + v) iterated 50x, B = AtA + g*DtD + a*I.  AtA is
rank-9 and C := g*DtD + a*I is circulant, so by Woodbury
    B^-1 = C^-1 - W2 U^T,   U = C^-1 A^T,  W2 = U S^-1,  S = I9 + A U.
C^-1 is applied as a banded (radius-32) circular convolution G; the
rank-9 correction uses q = U^T v.  All 8 cores run the same program
(SPMD, no collectives); core 0's output is returned.

Perf design (vs the fp32 baseline):
- every matmul is bf16 (4x PE throughput).  A1/B/I/ones have exact bf16
  entries; G is split hi+lo bf16 (two matmuls ~= fp16 operator
  precision), which kills the systematic operator-rounding error that a
  plain bf16 G would accumulate over 50 non-contracting iterations.
- At = A1 x + B x + (E - U) accumulates entirely in one PSUM bank on PE
  (an I-matmul folds the state term), so the soft-threshold reads PSUM
  directly and the En/At DVE adds leave the critical path.
- no halo columns: each banded operator's cross-column corner is applied
  as two column-shifted matmuls on the payload tile itself (out cols
  1:32 from src cols 0:31 plus the single wrap column), so nothing waits
  on halo maintenance copies.
- the tau-side state algebra is restructured so only two fused DVE ops
  (Dt = 2*Cx + tTW, P2n = -bankB - (Cx + S2)) sit before v; the state
  recurrences (tTW' = Cx + tTW - Wn, tAB' = En - Un, S2' = tAB' + tTW')
  run on GpSimd during the rank-9 tail of the same iteration.
- rank-9: Z1 = U2 (.) v as one bf16 multiply; the c-grouped sums go
  through Pool avg (window scales baked into W2 host-side, bf16 output,
  no fp32-only restriction -> no cast); a ones-matmul does the partition
  reduce AND the broadcast of q; c0 = B^-1 bA rides along as a 10th
  rank column with q10 = -1.
- x and v tiles are bf16; small bf16 filler matmuls keep the PE
  pipeline warm across its idle windows.

Vector layout: [128, 32] tiles, flat index i = k + 128*c at tile col c.
"""

import numpy as np

N = 4096
P = 128          # partitions
CCOL = 32        # payload columns; i = k + 128*c
RB = 32          # band radius of G
R9 = 9           # Woodbury rank
R10 = 10         # rank columns incl. the c0 slot
GAMMA = 10.0
ALPHA = 5.0
LAM = 1e-4
NIT = 50
NCORES = 8
USE_POOL = False  # Pool-avg grouped sums (False: tensor_reduce + cast)

# f32-column offsets inside the constant blob [128, BLOB_COLS].
# bf16 payloads are packed two-per-f32-column and bitcast on device.
_cur = 0
def _alloc(w):
    global _cur
    off = _cur
    _cur += w
    return off

OFF_A1M   = _alloc(64)    # gamma*(S+ - I) main, bf16 [128,128]
OFF_BM    = _alloc(64)    # gamma*(S- - I) main
OFF_ABM   = _alloc(64)    # gamma*(S+ + S- - 2I) main (A1_M + B_M)
OFF_IBF   = _alloc(64)    # identity (tAB fold into the At bank)
OFF_A1C   = _alloc(64)    # A1 corner (single element, padded)
OFF_BC    = _alloc(64)    # B corner
OFF_GHIM  = _alloc(64)    # C^-1 banded main, hi half
OFF_GLOM  = _alloc(64)    # lo half
OFF_GBLH  = _alloc(64)    # C^-1 left corner hi
OFF_GBLL  = _alloc(64)
OFF_GBHH  = _alloc(64)    # C^-1 right corner hi
OFF_GBHL  = _alloc(64)
OFF_ONES  = _alloc(64)    # all-ones bf16 (partition reduce + broadcast)
OFF_U2    = _alloc(144)   # U m-major: [k, m*32+c], bf16 [128, 288]
OFF_W2    = _alloc(160)   # [W2 | c0] c-major: [k, c*10+m], bf16
OFF_X0    = _alloc(16)    # x0 tile, bf16 [128, 32]
BLOB_COLS = _cur


def _bf16(x):
    x32 = np.asarray(x, np.float32)
    u = x32.view(np.uint32)
    r = ((u >> 16) + ((u >> 15) & 1)).astype(np.uint32) << 16
    return r.view(np.float32)


def _pack_bf16(arr):
    """[128, W] float (W even) -> [128, W//2] f32 with packed bf16 pairs."""
    a = _bf16(arr).view(np.uint32) >> 16
    lo, hi = a[:, 0::2], a[:, 1::2]
    return (lo | (hi << 16)).view(np.float32)


def _banded(h):
    """main/BL/BH lhsT pieces for kernel h (dict d -> coef), [128,128] each.
    lhsT[k, m]: contraction index k = input row, m = output row.
    main: within-column (shift d = k - m);
    BL: rhs = col c-1 view (shift d = k - 128 - m);
    BH: rhs = col c+1 view (shift d = k + 128 - m)."""
    B0 = np.zeros((P, P)); BL = np.zeros((P, P)); BH = np.zeros((P, P))
    for k in range(P):
        for m in range(P):
            if (k - m) in h:
                B0[k, m] = h[k - m]
            if (k - P - m) in h:
                BL[k, m] = h[k - P - m]
            if (k + P - m) in h:
                BH[k, m] = h[k + P - m]
    return B0, BL, BH


def _tile(vec):
    """[4096] -> [128, 32], i = k + 128*c."""
    return np.ascontiguousarray(np.asarray(vec).reshape(CCOL, P).T)


def host_constants(target, A, x0):
    """All f64 precompute; returns the [128, BLOB_COLS] f32 device blob."""
    A64 = np.asarray(A, np.float64)
    w = ALPHA + 2 * GAMMA * (1 - np.cos(2 * np.pi * np.arange(N // 2 + 1) / N))

    def C_inv(z):
        return np.fft.irfft(np.fft.rfft(z, axis=-1) / w, n=N, axis=-1)

    U = C_inv(A64).T                              # [N, 9]
    S = np.eye(R9) + A64 @ U
    W2 = U @ np.linalg.inv(S)                     # [N, 9]
    g = np.fft.irfft(1.0 / w, n=N)                # kernel of C^-1
    b = A64 @ np.asarray(target, np.float64)
    bA = b @ A64
    c0 = C_inv(bA) - W2 @ (U.T @ bA)              # B^-1 @ bA

    blob = np.zeros((P, BLOB_COLS), np.float32)

    def putb(off, arr):
        p = _pack_bf16(arr)
        blob[:p.shape[0], off:off + p.shape[1]] = p

    mA1 = _banded({-1: GAMMA, 0: -GAMMA})         # gamma*(S+ - I)
    mB = _banded({1: GAMMA, 0: -GAMMA})           # gamma*(S- - I)
    mG = _banded({d: g[d % N] for d in range(-RB, RB + 1)})
    Ghi = [_bf16(m).astype(np.float64) for m in mG]
    Glo = [m - h for m, h in zip(mG, Ghi)]

    putb(OFF_A1M, mA1[0]); putb(OFF_A1C, mA1[1])
    putb(OFF_BM, mB[0]); putb(OFF_BC, mB[2])
    putb(OFF_ABM, mA1[0] + mB[0])
    putb(OFF_GHIM, Ghi[0]); putb(OFF_GLOM, Glo[0])
    putb(OFF_GBLH, Ghi[1]); putb(OFF_GBLL, Glo[1])
    putb(OFF_GBHH, Ghi[2]); putb(OFF_GBHL, Glo[2])
    putb(OFF_IBF, np.eye(P))
    putb(OFF_ONES, np.ones((P, P)))

    # U2[k, m*32+c] = U[k + 128c, m] (m-major)
    putb(OFF_U2, U.reshape(CCOL, P, R9).transpose(1, 2, 0).reshape(P, R9 * CCOL))
    # W2e[k, c*10+m] = W2[k + 128c, m] for m<9;  c0[k + 128c] at m=9.
    # Pool avg divides the c-sum by 32 and the m-sum by 10: bake the
    # compensation into the constants (q10 = -1 skips the first pool).
    if USE_POOL:
        W2e = np.concatenate([W2 * (CCOL * R10), c0[:, None] * R10], axis=1)
    else:
        W2e = np.concatenate([W2, c0[:, None]], axis=1)
    putb(OFF_W2, W2e.reshape(CCOL, P, R10).transpose(1, 0, 2).reshape(P, CCOL * R10))

    putb(OFF_X0, _tile(np.asarray(x0, np.float64)))
    return np.ascontiguousarray(blob)


def build_nc():
    """Build and compile the Bacc graph (one core's program)."""
    from concourse import bacc, mybir, tile

    f32 = mybir.dt.float32
    bf16 = mybir.dt.bfloat16
    Relu = mybir.ActivationFunctionType.Relu
    Copy = mybir.ActivationFunctionType.Copy
    Alu = mybir.AluOpType
    Avg = mybir.PoolFunctionType.avg
    nc = bacc.Bacc(target_bir_lowering=False)

    blob_ext = nc.declare_dram_parameter("blob", [P, BLOB_COLS], f32, isOutput=False)
    out_ext = nc.declare_dram_parameter("out", [P, CCOL], f32, isOutput=True)

    with tile.TileContext(nc) as tc:
        with (
            tc.tile_pool(name="const", bufs=1) as cpool,
            tc.tile_pool(name="work", bufs=3) as wpool,
            tc.tile_pool(name="psum", bufs=1, space="PSUM") as ppool,
        ):
            cb = cpool.tile([P, BLOB_COLS], f32, tag="blob")
            nc.sync.dma_start(cb[:, :], blob_ext[:, :])

            def csb(off, wcols):
                """bf16 view of wcols f32 columns -> [128, 2*wcols] bf16"""
                return cb[:, off:off + wcols].bitcast(bf16)

            A1_M, A1_C = csb(OFF_A1M, 64), csb(OFF_A1C, 64)
            B_M, B_C = csb(OFF_BM, 64), csb(OFF_BC, 64)
            AB_M = csb(OFF_ABM, 64)
            I_bf = csb(OFF_IBF, 64)
            GHI_M, GLO_M = csb(OFF_GHIM, 64), csb(OFF_GLOM, 64)
            GBL_H, GBL_L = csb(OFF_GBLH, 64), csb(OFF_GBLL, 64)
            GBH_H, GBH_L = csb(OFF_GBHH, 64), csb(OFF_GBHL, 64)
            ones_bf = csb(OFF_ONES, 64)
            U2 = csb(OFF_U2, 144)                  # [128, 288] bf16
            U2_3d = U2.rearrange("k (m c) -> k m c", c=CCOL)
            W2e = csb(OFF_W2, 160)                 # [128, 320] bf16
            W2_3d = W2e.rearrange("k (c m) -> k c m", m=R10)

            def pool_avg(out_ap, in3d):
                """Grouped average via InstPool.  The input AP is lowered
                un-optimized with exactly 5 dims so the window (innermost
                dim) survives to codegen."""
                ap5 = in3d.unsqueeze(1).unsqueeze(1)
                return nc.vector.add_instruction(mybir.InstPool(
                    name=f"I-{nc.vector.bass.next_id()}",
                    func=Avg,
                    ins=[nc.vector.lower_ap(ap5, opt=False)],
                    outs=[nc.vector.lower_ap(out_ap)],
                ))

            def corner(bank, lhsT, src, shift, stop=False):
                """Cross-column corner of a banded circulant: out col c
                reads src col c+shift (mod 32), as two payload matmuls."""
                if shift == -1:
                    nc.tensor.matmul(bank[:, 1:CCOL], lhsT, src[:, 0:CCOL - 1],
                                     start=False, stop=False,
                                     skip_group_check=True)
                    nc.tensor.matmul(bank[:, 0:1], lhsT, src[:, CCOL - 1:CCOL],
                                     start=False, stop=stop,
                                     skip_group_check=True)
                else:
                    nc.tensor.matmul(bank[:, 0:CCOL - 1], lhsT, src[:, 1:CCOL],
                                     start=False, stop=False,
                                     skip_group_check=True)
                    nc.tensor.matmul(bank[:, CCOL - 1:CCOL], lhsT, src[:, 0:1],
                                     start=False, stop=stop,
                                     skip_group_check=True)

            # q partials [128, 10]: cols 0:9 rewritten each iteration,
            # col 9 = -1/128 so the ones-matmul yields q10 = -1 (c0 slot).
            Z1r = cpool.tile([P, 16], bf16, tag="Z1r")
            nc.vector.memset(Z1r[:, :], 0.0)
            nc.vector.memset(Z1r[:, R9:R9 + 1], -1.0 / P)

            Xh = csb(OFF_X0, 16)                   # [128, 32] bf16 state
            tABf = tABb = tTW = S2 = None

            for j in range(1, NIT + 1):
                first = (j == 1)
                last = (j == NIT)
                xm = Xh[:, 0:CCOL]

                # --- PE: At bank (A1 x + B x + tAB) and B bank ---
                bankAt = ppool.tile([P, CCOL], f32, tag="At")
                bankB = ppool.tile([P, CCOL], f32, tag="B")
                scr = ppool.tile([P, 64], f32, tag="scr")
                if first:
                    nc.tensor.matmul(bankAt[:, :], A1_M, xm, start=True, stop=False)
                    corner(bankAt, A1_C, xm, -1, stop=True)
                else:
                    nc.tensor.matmul(bankAt[:, :], AB_M, xm, start=True, stop=False)
                    corner(bankAt, A1_C, xm, -1)
                    corner(bankAt, B_C, xm, +1)
                    nc.tensor.matmul(bankAt[:, :], I_bf, tABb[:, :], start=False, stop=True)
                nc.tensor.matmul(bankB[:, :], B_M, xm, start=True, stop=False)
                corner(bankB, B_C, xm, +1, stop=True)
                # keep the PE pipeline warm while DVE runs the soft chain
                nc.tensor.matmul(scr[:, :], ones_bf, ones_bf[:, 0:64],
                                 start=True, stop=True)
                nc.tensor.matmul(scr[:, :], ones_bf, ones_bf[:, 0:64],
                                 start=True, stop=True)

                # --- Scalar: Cx = alpha*x ---
                if first:
                    Wn = wpool.tile([P, CCOL], f32, tag="Wn")
                    nc.scalar.activation(Wn[:, :], xm, Relu, bias=0.0, scale=ALPHA)
                else:
                    Cx = wpool.tile([P, CCOL], f32, tag="Cx")
                    nc.scalar.activation(Cx[:, :], xm, Copy, bias=0.0, scale=ALPHA)
                    # GpSimd: u3 = Cx + S2 feeds P2n
                    u3 = wpool.tile([P, CCOL], f32, tag="u3")
                    nc.gpsimd.tensor_add(u3[:, :], Cx[:, :], S2[:, :])

                # --- DVE: Dt, soft-threshold, v ---
                r1 = wpool.tile([P, CCOL], f32, tag="r1")
                Un = wpool.tile([P, CCOL], f32, tag="Un")
                vh = wpool.tile([P, CCOL], bf16, tag="vh")
                vm = vh[:, 0:CCOL]
                if not first:
                    Dt = wpool.tile([P, CCOL], f32, tag="Dt")
                    # Dt = 2*Cx + tTW  (= alpha*x + tau)
                    nc.vector.scalar_tensor_tensor(Dt[:, :], Cx[:, :], 2.0,
                                                   tTW[:, :], Alu.mult, Alu.add)
                nc.vector.tensor_scalar(r1[:, :], bankAt[:, :], -LAM, LAM,
                                        Alu.max, Alu.min)
                nc.vector.tensor_sub(Un[:, :], bankAt[:, :], r1[:, :])
                if first:
                    nc.vector.tensor_add(vm, Un[:, :], Wn[:, :])
                else:
                    P1 = wpool.tile([P, CCOL], f32, tag="P1")
                    P2n = wpool.tile([P, CCOL], f32, tag="P2n")
                    # P1 = relu(Dt) + Un;  P2n = -bankB - u3 = -(En + Tn)
                    nc.vector.scalar_tensor_tensor(P1[:, :], Dt[:, :], 0.0,
                                                   Un[:, :], Alu.max, Alu.add)
                    nc.vector.scalar_tensor_tensor(P2n[:, :], bankB[:, :], -1.0,
                                                   u3[:, :], Alu.mult,
                                                   Alu.subtract)
                    nc.vector.tensor_add(vm, P1[:, :], P2n[:, :])
                    # Scalar: Wn = relu(Dt) (state only)
                    Wn = wpool.tile([P, CCOL], f32, tag="Wn")
                    nc.scalar.activation(Wn[:, :], Dt[:, :], Relu,
                                         bias=0.0, scale=1.0)

                # --- PE: banded G apply (mains + halo-free corners) ---
                bankE = ppool.tile([P, CCOL], f32, tag="E")
                bankR = ppool.tile([P, 16], f32, tag="R")
                nc.tensor.matmul(bankE[:, :], GHI_M, vm, start=True, stop=False)
                nc.tensor.matmul(bankE[:, :], GLO_M, vm, start=False, stop=False)
                corner(bankE, GBL_H, vm, -1)
                corner(bankE, GBL_L, vm, -1)
                corner(bankE, GBH_H, vm, +1)
                corner(bankE, GBH_L, vm, +1, stop=True)

                # --- DVE: rank-9 q = U^T v ---
                Z1 = wpool.tile([P, R9 * CCOL], bf16, tag="Z1")
                z1_3d = Z1[:, :].rearrange("k (m c) -> k m c", c=CCOL)
                vb9 = vm.unsqueeze(1).broadcast_to([P, R9, CCOL])
                nc.vector.tensor_mul(z1_3d, U2_3d, vb9)
                if USE_POOL:
                    pool_avg(Z1r[:, 0:R9], z1_3d)
                    # PE: partition reduce + broadcast of q (and q10 = -1)
                    nc.tensor.matmul(bankR[:, 0:R10], ones_bf, Z1r[:, 0:R10],
                                     start=True, stop=True)
                else:
                    with nc.allow_low_precision(reason="q is consumed in bf16"):
                        nc.vector.tensor_reduce(Z1r[:, 0:R9], z1_3d,
                                                axis=mybir.AxisListType.X,
                                                op=Alu.add)
                    nc.tensor.matmul(bankR[:, 0:R10], ones_bf, Z1r[:, 0:R10],
                                     start=True, stop=True)
                nc.tensor.matmul(scr[:, :], ones_bf, ones_bf[:, 0:64],
                                 start=True, stop=True)

                # --- DVE: corr = W2e q (incl. -c0) and the x update ---
                qb = wpool.tile([P, 16], bf16, tag="qb")
                nc.vector.tensor_copy(qb[:, 0:R10], bankR[:, 0:R10])
                Z2 = wpool.tile([P, CCOL * R10], bf16, tag="Z2")
                corr = wpool.tile([P, CCOL], bf16, tag="corr")
                z2_3d = Z2[:, :].rearrange("k (c m) -> k c m", m=R10)
                rb = qb[:, 0:R10].unsqueeze(1).broadcast_to([P, CCOL, R10])
                nc.vector.tensor_mul(z2_3d, W2_3d, rb)
                if USE_POOL:
                    pool_avg(corr[:, :], z2_3d)
                else:
                    with nc.allow_low_precision(reason="x state is bf16"):
                        nc.vector.tensor_reduce(corr[:, :], z2_3d,
                                                axis=mybir.AxisListType.X,
                                                op=Alu.add)


                if not last:
                    Xn = wpool.tile([P, CCOL], bf16, tag="Xh")
                    nc.vector.tensor_sub(Xn[:, :], bankE[:, :], corr[:, :])
                    Xh = Xn
                    if not first:
                        # En = eta (state only, consumed by the GpSimd tail).
                        # Emitted after Xn: it fills the DVE-idle window at
                        # the next iteration's start and completes before the
                        # B-bank PSUM is rewritten, so the WAR dep is free.
                        En = wpool.tile([P, CCOL], f32, tag="En")
                        nc.vector.tensor_add(En[:, :], bankB[:, :], tABf[:, :])

                    # next-iteration state combos (GpSimd/Scalar, in the tail)
                    tABf_n = wpool.tile([P, CCOL], f32, tag="tABf")
                    tTW_n = wpool.tile([P, CCOL], f32, tag="tTW")
                    S2_n = wpool.tile([P, CCOL], f32, tag="S2")
                    tABb_n = wpool.tile([P, CCOL], bf16, tag="tABb")
                    if first:
                        nc.gpsimd.tensor_scalar_mul(tABf_n[:, :], Un[:, :], -1.0)
                        nc.gpsimd.tensor_scalar_mul(tTW_n[:, :], Wn[:, :], -1.0)
                    else:
                        nc.gpsimd.tensor_sub(tABf_n[:, :], En[:, :], Un[:, :])
                        # tTW' = Tn - Wn = (Cx + tTW) - Wn
                        tmpW = wpool.tile([P, CCOL], f32, tag="tmpW")
                        nc.gpsimd.tensor_sub(tmpW[:, :], Cx[:, :], Wn[:, :])
                        nc.gpsimd.tensor_add(tTW_n[:, :], tmpW[:, :], tTW[:, :])
                    nc.gpsimd.tensor_add(S2_n[:, :], tABf_n[:, :], tTW_n[:, :])
                    nc.scalar.activation(tABb_n[:, :], tABf_n[:, :], Copy,
                                         bias=0.0, scale=1.0)
                    tABf, tTW, S2, tABb = tABf_n, tTW_n, S2_n, tABb_n
                else:
                    Xout = wpool.tile([P, CCOL], f32, tag="Xout")
                    nc.vector.tensor_sub(Xout[:, :], bankE[:, :], corr[:, :])
                    nc.sync.dma_start(out_ext[:, :], Xout[:, :])
                    # touch the filler bank so dead-code elim keeps the
                    # pipeline-warming matmuls
                    scrk = wpool.tile([P, 1], f32, tag="scrk")
                    nc.vector.tensor_copy(scrk[:, :], scr[:, 0:1])
                    nc.sync.dma_start(out_ext[0:1, 0:1], scrk[0:1, 0:1])

    nc.compile()
    return nc


def kernel(**inputs):
    from concourse.bass_utils import run_bass_kernel_spmd

    target = np.asarray(inputs["target"], np.float32)
    A = np.asarray(inputs["A"], np.float32)
    x0 = np.asarray(inputs["x0"], np.float32)

    blob = host_constants(target, A, x0)
    nc = build_nc()
    in_maps = [{"blob": blob} for _ in range(NCORES)]
    res = run_bass_kernel_spmd(nc, in_maps, core_ids=list(range(NCORES)))
    out_tile = np.asarray(res.results[0]["out"], np.float32)
    return np.ascontiguousarray(out_tile.T.reshape(-1))
